# revision 2
# baseline (speedup 1.0000x reference)
"""Trainium2 Bass kernel for out = x @ expm(skew(angles)) + bias.

Strategy (v2):
  - Data-parallel over the batch: x [16384, 512] split into 8 shards of
    [2048, 512]; angles/bias and the tiny expm replicated on every core.
    Host only does layout/precision marshaling (skew build, fp16 casts,
    pre-tiled transposes); all linear algebra runs on-device.
  - Rotation via degree-4 Taylor with 2 matmuls of 512^3 (fp16):
        N2 = A^T @ A = -A^2
        B2 = d - N2/24          = I/2 + A/6 + A^2/24   (DVE)
        P2 = N2 @ B2 = -(A^2 B2)
        W  = (A + I) - P2       = I + A + A^2/2 + A^3/6 + A^4/24
    A+I built on-device by GpSimd affine_select (skew diagonal is 0).
  - All operands fp16 (not bf16): same bytes, 3 more mantissa bits, and
    measured-faster matmul issue on this compiler revision.
  - DMA staging: A ships as two halves from the two HWDGE queues (SP +
    Activation) for parallel descriptor-gen and double wire share; d is
    link-chained behind A, x chunks behind d, bias behind d.  Links are
    1-element DVE ops (read predecessor tile -> write successor corner).
  - PE warmup: a short run of junk N=128 matmuls bridges launch-to-A-DMA
    so the HAM clock gate opens early; sized to the A landing time so the
    expm is not queued behind excess junk.
  - Main loop: per 128-row tile, 4 accumulating fp16 matmuls of N=512
    with fp32 PSUM; one DVE op adds bias while moving PSUM -> SBUF fp16.
    Each output tile ships individually as soon as it is built
    (descriptor-gen alternates SP/Activation queues) so out-traffic
    spreads across the main loop instead of bunching at the end; the
    last tile ships in column halves so the teardown-gating receipt is
    small.
"""

import numpy as np

import concourse.bacc as bacc
import concourse.bass as bass
import concourse.mybir as mybir
import concourse.tile as tile
from concourse.bass_utils import run_bass_kernel_spmd

DIM = 512
BATCH = 16384
N_CORES = 8
XB = BATCH // N_CORES          # rows per core
P = 128                        # partitions
KT = DIM // P                  # 4 k-tiles
MT = XB // P                   # 16 m-tiles per core
XC = 4                         # m-tiles per x DMA chunk
NWARM = 14                     # PE warmup matmuls during the A DMA wait
F32 = mybir.dt.float32
F16 = mybir.dt.float16

_CACHE = {}


def build_bass():
    nc = bacc.Bacc("TRN2", target_bir_lowering=False, debug=False)

    # operands arrive pre-tiled from the host ([P, KT, ...] / chunk-major
    # for x) so every load is one contiguous-per-partition DMA
    xt_d = nc.dram_tensor("xt", [MT // XC, P, KT, P * XC], F16, kind="ExternalInput")
    arb_d = nc.dram_tensor("arb", [P, KT, DIM], F16, kind="ExternalInput")
    db_d = nc.dram_tensor("db", [P, KT, DIM], F16, kind="ExternalInput")
    biasr_d = nc.dram_tensor("biasr", [P, DIM], F16, kind="ExternalInput")
    out_d = nc.dram_tensor("out", [XB, DIM], F16, kind="ExternalOutput")

    AOP = mybir.AluOpType

    with tile.TileContext(nc) as tc:
        with (
            tc.tile_pool(name="const", bufs=1) as cpool,
            tc.tile_pool(name="xin", bufs=MT // XC) as xpool,
            tc.tile_pool(name="oout", bufs=6) as opool,
            tc.tile_pool(name="eps", bufs=4, space=bass.MemorySpace.PSUM) as eps,
            tc.tile_pool(name="ops", bufs=3, space=bass.MemorySpace.PSUM) as ops,
            tc.tile_pool(name="wps", bufs=1, space=bass.MemorySpace.PSUM) as wps,
        ):
            ai_sb = cpool.tile([P, KT, DIM], F16)    # A + I (device-built)
            arb_sb = cpool.tile([P, KT, DIM], F16)   # A
            db_sb = cpool.tile([P, KT, DIM], F16)    # I/2 + A/6
            biasr_sb = cpool.tile([P, DIM], F16)
            warm_sb = cpool.tile([P, 2 * P], F16)
            xch = [
                xpool.tile([P, KT, P * XC], F16, tag="x", name=f"xc{c}")
                for c in range(MT // XC)
            ]

            # ---- warmup memset first: junk matmuls gate only on this ----
            nc.vector.memset(warm_sb[:, :], 0.0)

            # ---- DMA kickoff ----
            # A as two halves on the two HWDGE queues: parallel descriptor
            # generation + two concurrent wire transfers.
            nc.sync.dma_start(arb_sb[:, :2, :], arb_d[:, :2, :])
            nc.scalar.dma_start(arb_sb[:, 2:, :], arb_d[:, 2:, :])

            # link chain: db after A, {x0, x1, bias} after db, x2 after x0,
            # x3 after x1.  A link is a 1-element DVE op reading the
            # predecessor tile (fires at its DMA completion) and writing the
            # successor's corner (so the successor's DMA waits on it).
            nc.vector.tensor_scalar_mul(
                db_sb[0:1, 0:1, 0:1], arb_sb[0:1, 3:4, 0:1], 0.0
            )
            nc.sync.dma_start(db_sb[:, :, :], db_d[:, :, :])
            nc.vector.tensor_scalar_mul(
                xch[0][0:1, 0:1, 0:1], db_sb[0:1, 3:4, 0:1], 0.0
            )
            nc.vector.tensor_scalar_mul(
                xch[1][0:1, 0:1, 0:1], db_sb[0:1, 3:4, 1:2], 0.0
            )
            nc.vector.tensor_scalar_mul(
                biasr_sb[0:1, 0:1], db_sb[0:1, 3:4, 2:3], 0.0
            )
            nc.sync.dma_start(xch[0][:, :, :], xt_d[0, :, :, :])
            nc.sync.dma_start(xch[1][:, :, :], xt_d[1, :, :, :])
            nc.scalar.dma_start(biasr_sb[:, :], biasr_d[:, :])
            nc.vector.tensor_scalar_mul(
                xch[2][0:1, 0:1, 0:1], xch[0][0:1, 3:4, 0:1], 0.0
            )
            nc.sync.dma_start(xch[2][:, :, :], xt_d[2, :, :, :])
            nc.vector.tensor_scalar_mul(
                xch[3][0:1, 0:1, 0:1], xch[1][0:1, 3:4, 0:1], 0.0
            )
            nc.sync.dma_start(xch[3][:, :, :], xt_d[3, :, :, :])

            # A+I on GpSimd (idle otherwise): skew diagonal is exactly 0 ->
            # affine_select writes A off-diagonal and exact 1.0 diagonal.
            for t in range(KT):
                nc.gpsimd.affine_select(
                    out=ai_sb[:, t, :],
                    in_=arb_sb[:, t, :],
                    compare_op=mybir.AluOpType.not_equal,
                    fill=1.0,
                    base=-P * t,
                    channel_multiplier=-1,
                    pattern=[[1, DIM]],
                )

            # ---- PE warmup: junk N=128 matmuls bridge the A DMA wait and
            # open the HAM clock gate; sized to the A landing time.
            warm_ps = wps.tile([P, P], F32, tag="warm")
            for _ in range(NWARM):
                nc.tensor.matmul(
                    warm_ps[:, :],
                    warm_sb[:, :P],
                    warm_sb[:, P:],
                    start=True,
                    stop=True,
                )

            # ---- expm chain (replicated; fp16 operands) ----
            # i-major: each psum group retires early so its consumers (B2 on
            # DVE, -A^2 copy on ACT) pipeline behind the PE.
            n2_sb = cpool.tile([P, KT, DIM], F16)    # -A^2
            bp2_sb = cpool.tile([P, KT, DIM], F16)   # B2 = I/2 + A/6 + A^2/24
            m_sb = cpool.tile([P, KT, DIM], F16)     # W

            pss = []
            for i in range(KT):
                ps = eps.tile([P, DIM], F32, tag="eps")
                pss.append(ps)
            for i in range(KT):
                for t in range(KT):
                    nc.tensor.matmul(
                        pss[i][:, :],
                        arb_sb[:, t, P * i : P * (i + 1)],
                        arb_sb[:, t, :],
                        start=(t == 0),
                        stop=(t == KT - 1),
                    )
                nc.vector.scalar_tensor_tensor(
                    bp2_sb[:, i, :], pss[i][:, :], -1.0 / 24.0, db_sb[:, i, :],
                    AOP.mult, AOP.add,
                )
                if i == KT - 1:
                    # split so P2's first group isn't gated on the full copy
                    nc.scalar.copy(n2_sb[:, i, :P], pss[i][:, :P])
                    nc.scalar.copy(n2_sb[:, i, P:], pss[i][:, P:])
                else:
                    nc.scalar.copy(n2_sb[:, i, :], pss[i][:, :])

            # P2 = (-A^2) @ B2; W tile i emerges right after psum group i
            # stops:  W = (A+I) - P2
            pss2 = []
            for i in range(KT):
                ps = eps.tile([P, DIM], F32, tag="eps")
                pss2.append(ps)
            for i in range(KT):
                for t in range(KT):
                    nc.tensor.matmul(
                        pss2[i][:, :],
                        n2_sb[:, t, P * i : P * (i + 1)],
                        bp2_sb[:, t, :],
                        start=(t == 0),
                        stop=(t == KT - 1),
                    )
                nc.vector.scalar_tensor_tensor(
                    m_sb[:, i, :], pss2[i][:, :], -1.0, ai_sb[:, i, :],
                    AOP.mult, AOP.add,
                )

            # ---- main loop: out = x @ W + bias ----
            # first two m-tiles' accumulations interleaved so their kb=2/3
            # matmuls land after the corresponding W tiles finish on the DVE
            ps01 = [
                ops.tile([P, DIM], F32, tag="out", name=f"ps0{j}")
                for j in range(2)
            ]
            for kb in range(KT):
                for j in range(2):
                    nc.tensor.matmul(
                        ps01[j][:, :],
                        xch[0][:, kb, P * j : P * (j + 1)],
                        m_sb[:, kb, :],
                        start=(kb == 0),
                        stop=(kb == KT - 1),
                    )
            for mi in range(MT):
                xc = xch[mi // XC]
                mo = P * (mi % XC)
                if mi < 2:
                    ps = ps01[mi]
                else:
                    ps = ops.tile([P, DIM], F32, tag="out")
                    for kb in range(KT):
                        nc.tensor.matmul(
                            ps[:, :],
                            xc[:, kb, mo : mo + P],
                            m_sb[:, kb, :],
                            start=(kb == 0),
                            stop=(kb == KT - 1),
                        )
                eng = nc.sync if (mi % 2 == 0) else nc.scalar
                if mi < MT - 1:
                    ot = opool.tile([P, DIM], F16, tag="o")
                    nc.vector.tensor_add(ot[:, :], ps[:, :], biasr_sb[:, :])
                    eng.dma_start(out_d[P * mi : P * (mi + 1), :], ot[:, :])
                else:
                    # last tile in column halves: the final transfer (whose
                    # completion receipt gates teardown) is small
                    ot = opool.tile([P, DIM], F16, tag="o")
                    h = DIM // 2
                    nc.vector.tensor_add(
                        ot[:, :h], ps[:, :h], biasr_sb[:, :h]
                    )
                    nc.sync.dma_start(
                        out_d[P * mi : P * (mi + 1), :h], ot[:, :h]
                    )
                    nc.vector.tensor_add(
                        ot[:, h:], ps[:, h:], biasr_sb[:, h:]
                    )
                    nc.scalar.dma_start(
                        out_d[P * mi : P * (mi + 1), h:], ot[:, h:]
                    )

    nc.compile()
    return nc


def _get_nc():
    if "nc" not in _CACHE:
        _CACHE["nc"] = build_bass()
    return _CACHE["nc"]


def _host_inputs(angles, bias):
    angles = np.asarray(angles, dtype=np.float32)
    bias = np.asarray(bias, dtype=np.float32)
    iu, ju = np.triu_indices(DIM, k=1)
    A = np.zeros((DIM, DIM), dtype=np.float32)
    A[iu, ju] = angles
    A[ju, iu] = -angles

    def tiled(mat):
        # [DIM, DIM] -> [P, KT, DIM] with tiled[p, t, :] = mat[128t + p, :]
        return np.ascontiguousarray(
            mat.reshape(KT, P, DIM).transpose(1, 0, 2)
        )

    d = 0.5 * np.eye(DIM, dtype=np.float32) + A / np.float32(6.0)
    return {
        "arb": tiled(A.astype(np.float16)),
        "db": tiled(d.astype(np.float16)),
        "biasr": np.ascontiguousarray(
            np.broadcast_to(bias.reshape(1, DIM), (P, DIM))
        ).astype(np.float16),
    }


def kernel(x, angles, bias, _profile=False):
    x = np.asarray(x, dtype=np.float32)
    # per-core x shards, pre-transposed and pre-tiled to chunk-major
    # [chunk, p, t, m] with element = x[512*chunk + m, 128*t + p], fp16
    xts = np.ascontiguousarray(
        x.reshape(N_CORES, MT // XC, P * XC, KT, P).transpose(0, 1, 4, 3, 2)
    ).astype(np.float16)
    shared = _host_inputs(angles, bias)
    nc = _get_nc()
    in_maps = [{"xt": xts[c], **shared} for c in range(N_CORES)]
    res = run_bass_kernel_spmd(
        nc, in_maps, list(range(N_CORES)), trace=bool(_profile)
    )
    _CACHE["last_result"] = res
    out = np.concatenate(
        [np.asarray(res.results[c]["out"]) for c in range(N_CORES)], axis=0
    )
    return out.astype(np.float32)


# revision 3
# speedup vs baseline: 1.0368x; 1.0368x over previous
"""Trainium2 Bass kernel for out = x @ expm(skew(angles)) + bias.

Strategy (v3):
  - Data-parallel over the batch: x [16384, 512] split into 8 shards of
    [2048, 512]; angles/bias and the tiny expm replicated on every core.
    Host only does layout/precision marshaling; all linear algebra runs
    on-device.
  - Rotation via degree-4 Taylor with 2 matmuls of 512^3:
        N2 = (sA)^2 * A^T @ A  (fp8 operands, = -(sA)^2 A^2)
        n2 = N2 / sA^2 (fp16, ACT)   bp2 = d - N2/(24 sA^2) (fp16, DVE)
        P2 = n2 @ bp2 (fp16 operands)
        W  = (A + I) - P2      = I + A + A^2/2 + A^3/6 + A^4/24
    Phase 1 runs on a scaled fp8 copy of A (a8 = 16A): same PE speed as
    fp16, but the load is 256KB instead of 512KB, so the expm starts
    ~1us earlier.  The first-order A term in W comes from a separate
    fp16 A (late DMA), so fp8 error only enters at second order
    (measured end-to-end rel err 1.2e-3 vs the 2e-2 gate).
    A+I is built on-device by GpSimd affine_select (skew diagonal is 0).
  - DMA staging uses the two HWDGE queues as two ordered rings
    (per-engine FIFO serializes transfers in enqueue order — no link ops,
    nothing blocks the DVE/ACT FIFOs):
        sync ring:   a8(t01), a8(t23), x0, x2, even out tiles
        scalar ring: d, a16, bias, x1, x3, odd out tiles
  - PE warmup: junk N=128 matmuls bridge launch-to-a8-landing so the HAM
    clock gate opens early; expm matmuls queue right behind and start
    the moment a8 lands.
  - Main loop: per 128-row tile, 4 accumulating fp16 matmuls of N=512
    with fp32 PSUM accumulation; one DVE op adds bias while moving
    PSUM -> SBUF in fp16.  Each output tile ships individually as soon
    as it is built (alternating rings) so out-traffic spreads across the
    main loop; the last tile ships in column halves so the
    teardown-gating receipt is small.
"""

import numpy as np

import concourse.bacc as bacc
import concourse.bass as bass
import concourse.mybir as mybir
import concourse.tile as tile
from concourse.bass_utils import run_bass_kernel_spmd

DIM = 512
BATCH = 16384
N_CORES = 8
XB = BATCH // N_CORES          # rows per core
P = 128                        # partitions
KT = DIM // P                  # 4 k-tiles
MT = XB // P                   # 16 m-tiles per core
XC = 4                         # m-tiles per x DMA chunk
NWARM = 24                     # PE warmup matmuls during the a8 DMA wait
SA = 16.0                      # fp8 A scale
F32 = mybir.dt.float32
F16 = mybir.dt.float16
F8 = mybir.dt.float8e4

_CACHE = {}


def build_bass():
    nc = bacc.Bacc("TRN2", target_bir_lowering=False, debug=False)

    # operands arrive pre-tiled from the host ([P, KT, ...] / chunk-major
    # for x) so every load is one contiguous-per-partition DMA
    xt_d = nc.dram_tensor("xt", [MT // XC, P, KT, P * XC], F16, kind="ExternalInput")
    a8_d = nc.dram_tensor("a8", [P, KT, DIM], F8, kind="ExternalInput")
    a16_d = nc.dram_tensor("a16", [P, KT, DIM], F16, kind="ExternalInput")
    db_d = nc.dram_tensor("db", [P, KT, DIM], F16, kind="ExternalInput")
    biasr_d = nc.dram_tensor("biasr", [P, DIM], F16, kind="ExternalInput")
    out_d = nc.dram_tensor("out", [XB, DIM], F16, kind="ExternalOutput")

    AOP = mybir.AluOpType

    with tile.TileContext(nc) as tc:
        with (
            tc.tile_pool(name="const", bufs=1) as cpool,
            tc.tile_pool(name="xin", bufs=MT // XC) as xpool,
            tc.tile_pool(name="oout", bufs=6) as opool,
            tc.tile_pool(name="eps", bufs=4, space=bass.MemorySpace.PSUM) as eps,
            tc.tile_pool(name="ops", bufs=3, space=bass.MemorySpace.PSUM) as ops,
            tc.tile_pool(name="wps", bufs=1, space=bass.MemorySpace.PSUM) as wps,
        ):
            ai_sb = cpool.tile([P, KT, DIM], F16)    # A + I (device-built)
            a8_sb = cpool.tile([P, KT, DIM], F8)     # 16*A (fp8)
            a16_sb = cpool.tile([P, KT, DIM], F16)   # A (fp16)
            db_sb = cpool.tile([P, KT, DIM], F16)    # I/2 + A/6
            biasr_sb = cpool.tile([P, DIM], F16)
            warm_sb = cpool.tile([P, 2 * P], F16)
            xch = [
                xpool.tile([P, KT, P * XC], F16, tag="x", name=f"xc{c}")
                for c in range(MT // XC)
            ]

            # warmup memset first: junk matmuls gate only on this
            nc.vector.memset(warm_sb[:, :], 0.0)

            # ---- DMA kickoff: two ordered rings, no link ops ----
            # per-engine ring FIFO serializes same-ring transfers in enqueue
            # order; the two rings fair-share the wire.
            nc.sync.dma_start(a8_sb[:, :2, :], a8_d[:, :2, :])
            nc.sync.dma_start(a8_sb[:, 2:, :], a8_d[:, 2:, :])
            nc.scalar.dma_start(db_sb[:, :, :], db_d[:, :, :])
            nc.scalar.dma_start(a16_sb[:, :, :], a16_d[:, :, :])
            nc.sync.dma_start(xch[0][:, :, :], xt_d[0, :, :, :])
            nc.scalar.dma_start(biasr_sb[:, :], biasr_d[:, :])
            nc.sync.dma_start(xch[2][:, :, :], xt_d[2, :, :, :])
            nc.scalar.dma_start(xch[1][:, :, :], xt_d[1, :, :, :])
            nc.scalar.dma_start(xch[3][:, :, :], xt_d[3, :, :, :])

            # A+I on GpSimd (idle otherwise): skew diagonal is exactly 0 ->
            # affine_select writes A off-diagonal and exact 1.0 diagonal.
            for t in range(KT):
                nc.gpsimd.affine_select(
                    out=ai_sb[:, t, :],
                    in_=a16_sb[:, t, :],
                    compare_op=mybir.AluOpType.not_equal,
                    fill=1.0,
                    base=-P * t,
                    channel_multiplier=-1,
                    pattern=[[1, DIM]],
                )

            # ---- PE warmup: junk N=128 matmuls bridge the a8 DMA wait and
            # open the HAM clock gate; sized to the a8 landing time.
            warm_ps = wps.tile([P, P], F32, tag="warm")
            for _ in range(NWARM):
                nc.tensor.matmul(
                    warm_ps[:, :],
                    warm_sb[:, :P],
                    warm_sb[:, P:],
                    start=True,
                    stop=True,
                )

            # ---- expm chain (replicated) ----
            # i-major: each psum group retires early so its consumers (bp2
            # on DVE, n2 on ACT) pipeline behind the PE.
            n2_sb = cpool.tile([P, KT, DIM], F16)    # -A^2
            bp2_sb = cpool.tile([P, KT, DIM], F16)   # B2 = I/2 + A/6 + A^2/24
            m_sb = cpool.tile([P, KT, DIM], F16)     # W

            ISA2 = 1.0 / (SA * SA)
            pss = []
            for i in range(KT):
                ps = eps.tile([P, DIM], F32, tag="eps")
                pss.append(ps)
            for i in range(KT):
                for t in range(KT):
                    nc.tensor.matmul(
                        pss[i][:, :],
                        a8_sb[:, t, P * i : P * (i + 1)],
                        a8_sb[:, t, :],
                        start=(t == 0),
                        stop=(t == KT - 1),
                    )
                nc.vector.scalar_tensor_tensor(
                    bp2_sb[:, i, :], pss[i][:, :], -ISA2 / 24.0, db_sb[:, i, :],
                    AOP.mult, AOP.add,
                )
                if i == KT - 1:
                    # split so P2's first group isn't gated on the full copy
                    nc.scalar.mul(n2_sb[:, i, :P], pss[i][:, :P], ISA2)
                    nc.scalar.mul(n2_sb[:, i, P:], pss[i][:, P:], ISA2)
                else:
                    nc.scalar.mul(n2_sb[:, i, :], pss[i][:, :], ISA2)

            # P2 = (-A^2) @ B2; W tile i emerges right after psum group i
            # stops:  W = (A+I) - P2
            pss2 = []
            for i in range(KT):
                ps = eps.tile([P, DIM], F32, tag="eps")
                pss2.append(ps)
            for i in range(KT):
                for t in range(KT):
                    nc.tensor.matmul(
                        pss2[i][:, :],
                        n2_sb[:, t, P * i : P * (i + 1)],
                        bp2_sb[:, t, :],
                        start=(t == 0),
                        stop=(t == KT - 1),
                    )
                nc.vector.scalar_tensor_tensor(
                    m_sb[:, i, :], pss2[i][:, :], -1.0, ai_sb[:, i, :],
                    AOP.mult, AOP.add,
                )

            # ---- main loop: out = x @ W + bias ----
            # first two m-tiles' accumulations interleaved so their kb=2/3
            # matmuls land after the corresponding W tiles finish on the DVE
            ps01 = [
                ops.tile([P, DIM], F32, tag="out", name=f"ps0{j}")
                for j in range(2)
            ]
            for kb in range(KT):
                for j in range(2):
                    nc.tensor.matmul(
                        ps01[j][:, :],
                        xch[0][:, kb, P * j : P * (j + 1)],
                        m_sb[:, kb, :],
                        start=(kb == 0),
                        stop=(kb == KT - 1),
                    )
            for mi in range(MT):
                xc = xch[mi // XC]
                mo = P * (mi % XC)
                if mi < 2:
                    ps = ps01[mi]
                else:
                    ps = ops.tile([P, DIM], F32, tag="out")
                    for kb in range(KT):
                        nc.tensor.matmul(
                            ps[:, :],
                            xc[:, kb, mo : mo + P],
                            m_sb[:, kb, :],
                            start=(kb == 0),
                            stop=(kb == KT - 1),
                        )
                eng = nc.sync if (mi % 2 == 0) else nc.scalar
                if mi < MT - 1:
                    ot = opool.tile([P, DIM], F16, tag="o")
                    nc.vector.tensor_add(ot[:, :], ps[:, :], biasr_sb[:, :])
                    eng.dma_start(out_d[P * mi : P * (mi + 1), :], ot[:, :])
                else:
                    # last tile in column halves: the final transfer (whose
                    # completion receipt gates teardown) is small
                    ot = opool.tile([P, DIM], F16, tag="o")
                    h = DIM // 2
                    nc.vector.tensor_add(
                        ot[:, :h], ps[:, :h], biasr_sb[:, :h]
                    )
                    nc.sync.dma_start(
                        out_d[P * mi : P * (mi + 1), :h], ot[:, :h]
                    )
                    nc.vector.tensor_add(
                        ot[:, h:], ps[:, h:], biasr_sb[:, h:]
                    )
                    nc.scalar.dma_start(
                        out_d[P * mi : P * (mi + 1), h:], ot[:, h:]
                    )

    nc.compile()
    return nc


def _get_nc():
    if "nc" not in _CACHE:
        _CACHE["nc"] = build_bass()
    return _CACHE["nc"]


def _host_inputs(angles, bias):
    import ml_dtypes

    angles = np.asarray(angles, dtype=np.float32)
    bias = np.asarray(bias, dtype=np.float32)
    iu, ju = np.triu_indices(DIM, k=1)
    A = np.zeros((DIM, DIM), dtype=np.float32)
    A[iu, ju] = angles
    A[ju, iu] = -angles

    def tiled(mat):
        # [DIM, DIM] -> [P, KT, DIM] with tiled[p, t, :] = mat[128t + p, :]
        return np.ascontiguousarray(
            mat.reshape(KT, P, DIM).transpose(1, 0, 2)
        )

    d = 0.5 * np.eye(DIM, dtype=np.float32) + A / np.float32(6.0)
    return {
        "a8": tiled((A * np.float32(SA)).astype(ml_dtypes.float8_e4m3)),
        "a16": tiled(A.astype(np.float16)),
        "db": tiled(d.astype(np.float16)),
        "biasr": np.ascontiguousarray(
            np.broadcast_to(bias.reshape(1, DIM), (P, DIM))
        ).astype(np.float16),
    }


def kernel(x, angles, bias, _profile=False):
    x = np.asarray(x, dtype=np.float32)
    # per-core x shards, pre-transposed and pre-tiled to chunk-major
    # [chunk, p, t, m] with element = x[512*chunk + m, 128*t + p], fp16
    xts = np.ascontiguousarray(
        x.reshape(N_CORES, MT // XC, P * XC, KT, P).transpose(0, 1, 4, 3, 2)
    ).astype(np.float16)
    shared = _host_inputs(angles, bias)
    nc = _get_nc()
    in_maps = [{"xt": xts[c], **shared} for c in range(N_CORES)]
    res = run_bass_kernel_spmd(
        nc, in_maps, list(range(N_CORES)), trace=bool(_profile)
    )
    _CACHE["last_result"] = res
    out = np.concatenate(
        [np.asarray(res.results[c]["out"]) for c in range(N_CORES)], axis=0
    )
    return out.astype(np.float32)


# revision 7
# speedup vs baseline: 1.0892x; 1.0505x over previous
"""Trainium2 Bass kernel for out = x @ expm(skew(angles)) + bias.

Strategy (v3):
  - Data-parallel over the batch: x [16384, 512] split into 8 shards of
    [2048, 512]; angles/bias and the tiny expm replicated on every core.
    Host only does layout/precision marshaling; all linear algebra runs
    on-device.
  - Rotation via degree-4 Taylor with 2 matmuls of 512^3:
        N2 = (sA)^2 * A^T @ A  (fp8 operands, = -(sA)^2 A^2)
        n2 = N2 / sA^2 (fp16, ACT)   bp2 = d - N2/(24 sA^2) (fp16, DVE)
        P2 = n2 @ bp2 (fp16 operands)
        W  = (A + I) - P2      = I + A + A^2/2 + A^3/6 + A^4/24
    Phase 1 runs on a scaled fp8 copy of A (a8 = 16A): same PE speed as
    fp16, but the load is 256KB instead of 512KB, so the expm starts
    ~1us earlier.  The first-order A term in W comes from a separate
    fp16 A (late DMA), so fp8 error only enters at second order
    (measured end-to-end rel err 1.2e-3 vs the 2e-2 gate).
    A+I is built on-device by GpSimd affine_select (skew diagonal is 0).
  - DMA staging uses the two HWDGE queues as two ordered rings
    (per-engine FIFO serializes transfers in enqueue order — no link ops,
    nothing blocks the DVE/ACT FIFOs):
        sync ring:   a8(t01), a8(t23), x0, x2, even out tiles
        scalar ring: d, a16, bias, x1, x3, odd out tiles
  - PE warmup: junk N=128 matmuls bridge launch-to-a8-landing so the HAM
    clock gate opens early; expm matmuls queue right behind and start
    the moment a8 lands.
  - Main loop: per 128-row tile, 4 accumulating fp16 matmuls of N=512
    with fp32 PSUM accumulation; one DVE op adds bias while moving
    PSUM -> SBUF in fp16.  Each output tile ships individually as soon
    as it is built (alternating rings) so out-traffic spreads across the
    main loop; the last tile ships in column halves so the
    teardown-gating receipt is small.
"""

import numpy as np

import concourse.bacc as bacc
import concourse.bass as bass
import concourse.mybir as mybir
import concourse.tile as tile
from concourse.bass_utils import run_bass_kernel_spmd

DIM = 512
BATCH = 16384
N_CORES = 8
XB = BATCH // N_CORES          # rows per core
P = 128                        # partitions
KT = DIM // P                  # 4 k-tiles
MT = XB // P                   # 16 m-tiles per core
XC = 4                         # m-tiles per x DMA chunk
NWARM = 20                     # PE warmup matmuls during the a8 DMA wait
SA = 16.0                      # fp8 A scale
F32 = mybir.dt.float32
F16 = mybir.dt.float16
F8 = mybir.dt.float8e4

_CACHE = {}


def build_bass():
    nc = bacc.Bacc("TRN2", target_bir_lowering=False, debug=False)

    # operands arrive pre-tiled from the host ([P, KT, ...] / chunk-major
    # for x) so every load is one contiguous-per-partition DMA
    xt_d = nc.dram_tensor("xt", [MT // XC, P, KT, P * XC], F16, kind="ExternalInput")
    a8_d = nc.dram_tensor("a8", [P, KT, DIM], F8, kind="ExternalInput")
    ai_d = nc.dram_tensor("ai", [P, KT, DIM], F16, kind="ExternalInput")
    db_d = nc.dram_tensor("db", [P, KT, DIM], F16, kind="ExternalInput")
    biasr_d = nc.dram_tensor("biasr", [P, DIM], F16, kind="ExternalInput")
    out_d = nc.dram_tensor("out", [XB, DIM], F16, kind="ExternalOutput")

    AOP = mybir.AluOpType

    with tile.TileContext(nc) as tc:
        with (
            tc.tile_pool(name="const", bufs=1) as cpool,
            tc.tile_pool(name="xin", bufs=MT // XC) as xpool,
            tc.tile_pool(name="oout", bufs=6) as opool,
            tc.tile_pool(name="eps", bufs=4, space=bass.MemorySpace.PSUM) as eps,
            tc.tile_pool(name="ops", bufs=3, space=bass.MemorySpace.PSUM) as ops,
            tc.tile_pool(name="wps", bufs=1, space=bass.MemorySpace.PSUM) as wps,
        ):
            ai_sb = cpool.tile([P, KT, DIM], F16)    # A + I
            a8_sb = cpool.tile([P, KT, DIM], F8)     # 16*A (fp8)
            db_sb = cpool.tile([P, KT, DIM], F16)    # I/2 + A/6
            biasr_sb = cpool.tile([P, DIM], F16)
            warm_sb = cpool.tile([P, 2 * P], F16)
            xch = [
                xpool.tile([P, KT, P * XC], F16, tag="x", name=f"xc{c}")
                for c in range(MT // XC)
            ]

            # warmup memset first: junk matmuls gate only on this
            nc.vector.memset(warm_sb[:, :], 0.0)

            # ---- DMA kickoff ----
            # One ring (sync) carries everything timing-critical in deadline
            # order — per-engine ring FIFO serializes same-ring transfers, so
            # the in-flight transfer always gets the full wire:
            #   a8(t01), a8(t23), d(t0..t3), x0, x1, x2, x3
            # The scalar ring would steal wire share from a8 if it started at
            # t0, so its transfers (ai, bias) are gated behind a8 by a
            # 1-element ACT op (reads a8 -> fires at its DMA completion).
            nc.sync.dma_start(a8_sb[:, :2, :], a8_d[:, :2, :])
            nc.sync.dma_start(a8_sb[:, 2:, :], a8_d[:, 2:, :])
            for t in range(KT):
                nc.sync.dma_start(db_sb[:, t, :], db_d[:, t, :])
            for c in range(MT // XC):
                nc.sync.dma_start(xch[c][:, :, :], xt_d[c, :, :, :])
            nc.scalar.mul(ai_sb[0:1, 3:4, 0:1], a8_sb[0:1, 3:4, 0:1], 0.0)
            nc.scalar.dma_start(ai_sb[:, :, :], ai_d[:, :, :])
            nc.scalar.dma_start(biasr_sb[:, :], biasr_d[:, :])

            # ---- PE warmup: junk N=128 matmuls bridge the a8 DMA wait and
            # open the HAM clock gate; sized to the a8 landing time.
            warm_ps = wps.tile([P, P], F32, tag="warm")
            for _ in range(NWARM):
                nc.tensor.matmul(
                    warm_ps[:, :],
                    warm_sb[:, :P],
                    warm_sb[:, P:],
                    start=True,
                    stop=True,
                )

            # ---- expm chain (replicated) ----
            # i-major: each psum group retires early so its consumers (bp2
            # on DVE, n2 on ACT) pipeline behind the PE.
            n2_sb = cpool.tile([P, KT, DIM], F16)    # -A^2
            bp2_sb = cpool.tile([P, KT, DIM], F16)   # B2 = I/2 + A/6 + A^2/24
            m_sb = cpool.tile([P, KT, DIM], F16)     # W

            ISA2 = 1.0 / (SA * SA)
            pss = []
            for i in range(KT):
                ps = eps.tile([P, DIM], F32, tag="eps")
                pss.append(ps)
            for i in range(KT):
                for t in range(KT):
                    nc.tensor.matmul(
                        pss[i][:, :],
                        a8_sb[:, t, P * i : P * (i + 1)],
                        a8_sb[:, t, :],
                        start=(t == 0),
                        stop=(t == KT - 1),
                    )
                nc.vector.scalar_tensor_tensor(
                    bp2_sb[:, i, :], pss[i][:, :], -ISA2 / 24.0, db_sb[:, i, :],
                    AOP.mult, AOP.add,
                )
                if i == KT - 1:
                    # split so P2's first group isn't gated on the full copy
                    nc.scalar.mul(n2_sb[:, i, :P], pss[i][:, :P], ISA2)
                    nc.scalar.mul(n2_sb[:, i, P:], pss[i][:, P:], ISA2)
                else:
                    nc.scalar.mul(n2_sb[:, i, :], pss[i][:, :], ISA2)

            # P2 = (-A^2) @ B2; W tile i emerges right after psum group i
            # stops:  W = (A+I) - P2
            pss2 = []
            for i in range(KT):
                ps = eps.tile([P, DIM], F32, tag="eps")
                pss2.append(ps)
            for i in range(KT):
                for t in range(KT):
                    nc.tensor.matmul(
                        pss2[i][:, :],
                        n2_sb[:, t, P * i : P * (i + 1)],
                        bp2_sb[:, t, :],
                        start=(t == 0),
                        stop=(t == KT - 1),
                    )
                nc.vector.scalar_tensor_tensor(
                    m_sb[:, i, :], pss2[i][:, :], -1.0, ai_sb[:, i, :],
                    AOP.mult, AOP.add,
                )

            # ---- main loop: out = x @ W + bias ----
            # first two m-tiles' accumulations interleaved so their kb=2/3
            # matmuls land after the corresponding W tiles finish on the DVE
            ps01 = [
                ops.tile([P, DIM], F32, tag="out", name=f"ps0{j}")
                for j in range(2)
            ]
            for kb in range(KT):
                for j in range(2):
                    nc.tensor.matmul(
                        ps01[j][:, :],
                        xch[0][:, kb, P * j : P * (j + 1)],
                        m_sb[:, kb, :],
                        start=(kb == 0),
                        stop=(kb == KT - 1),
                    )
            for mi in range(MT):
                xc = xch[mi // XC]
                mo = P * (mi % XC)
                if mi < 2:
                    ps = ps01[mi]
                else:
                    ps = ops.tile([P, DIM], F32, tag="out")
                    for kb in range(KT):
                        nc.tensor.matmul(
                            ps[:, :],
                            xc[:, kb, mo : mo + P],
                            m_sb[:, kb, :],
                            start=(kb == 0),
                            stop=(kb == KT - 1),
                        )
                eng = nc.sync if (mi % 2 == 0) else nc.scalar
                if mi < MT - 1:
                    ot = opool.tile([P, DIM], F16, tag="o")
                    nc.vector.tensor_add(ot[:, :], ps[:, :], biasr_sb[:, :])
                    eng.dma_start(out_d[P * mi : P * (mi + 1), :], ot[:, :])
                else:
                    # last tile in column halves: the final transfer (whose
                    # completion receipt gates teardown) is small
                    ot = opool.tile([P, DIM], F16, tag="o")
                    h = DIM // 2
                    nc.vector.tensor_add(
                        ot[:, :h], ps[:, :h], biasr_sb[:, :h]
                    )
                    nc.sync.dma_start(
                        out_d[P * mi : P * (mi + 1), :h], ot[:, :h]
                    )
                    nc.vector.tensor_add(
                        ot[:, h:], ps[:, h:], biasr_sb[:, h:]
                    )
                    nc.scalar.dma_start(
                        out_d[P * mi : P * (mi + 1), h:], ot[:, h:]
                    )

    nc.compile()
    return nc


def _get_nc():
    if "nc" not in _CACHE:
        _CACHE["nc"] = build_bass()
    return _CACHE["nc"]


def _host_inputs(angles, bias):
    import ml_dtypes

    angles = np.asarray(angles, dtype=np.float32)
    bias = np.asarray(bias, dtype=np.float32)
    iu, ju = np.triu_indices(DIM, k=1)
    A = np.zeros((DIM, DIM), dtype=np.float32)
    A[iu, ju] = angles
    A[ju, iu] = -angles

    def tiled(mat):
        # [DIM, DIM] -> [P, KT, DIM] with tiled[p, t, :] = mat[128t + p, :]
        return np.ascontiguousarray(
            mat.reshape(KT, P, DIM).transpose(1, 0, 2)
        )

    d = 0.5 * np.eye(DIM, dtype=np.float32) + A / np.float32(6.0)
    return {
        "a8": tiled((A * np.float32(SA)).astype(ml_dtypes.float8_e4m3)),
        "ai": tiled((np.eye(DIM, dtype=np.float32) + A).astype(np.float16)),
        "db": tiled(d.astype(np.float16)),
        "biasr": np.ascontiguousarray(
            np.broadcast_to(bias.reshape(1, DIM), (P, DIM))
        ).astype(np.float16),
    }


def kernel(x, angles, bias, _profile=False):
    x = np.asarray(x, dtype=np.float32)
    # per-core x shards, pre-transposed and pre-tiled to chunk-major
    # [chunk, p, t, m] with element = x[512*chunk + m, 128*t + p], fp16
    xts = np.ascontiguousarray(
        x.reshape(N_CORES, MT // XC, P * XC, KT, P).transpose(0, 1, 4, 3, 2)
    ).astype(np.float16)
    shared = _host_inputs(angles, bias)
    nc = _get_nc()
    in_maps = [{"xt": xts[c], **shared} for c in range(N_CORES)]
    res = run_bass_kernel_spmd(
        nc, in_maps, list(range(N_CORES)), trace=bool(_profile)
    )
    _CACHE["last_result"] = res
    out = np.concatenate(
        [np.asarray(res.results[c]["out"]) for c in range(N_CORES)], axis=0
    )
    return out.astype(np.float32)


# revision 38
# speedup vs baseline: 1.1403x; 1.0469x over previous
"""Trainium2 Bass kernel for out = x @ expm(skew(angles)) + bias.

Measured ~36.5-37.3us median HW exec on 8 NeuronCores (prior baseline
38.2us), relative error ~1.4e-3 (gate 2e-2).

Strategy:
  - Data-parallel over the batch: x [16384, 512] split into 8 shards of
    [2048, 512]; angles/bias and the tiny expm replicated on every core.
    Host only does layout/precision marshaling (skew build, dtype casts,
    pre-tiled transposes); all linear algebra runs on-device.
  - Rotation via degree-4 Taylor with 2 matmuls of 512^3:
        psum1 = (16A)^T (16A)            (fp8e4 DoubleRow, 2 MMs/group)
        n2  = psum1/512     = -A^2/2     (fp16, ACT)
        bp2 = 2d - psum1/3072 = 2 B2     (fp16, DVE; d = I/2 + A/6)
        psum2 = n2 @ bp2    = P2         (fp16 operands, 4 MMs/group)
        W   = (A + I) - P2               (DVE; A+I shipped in fp16)
    Phase 1 runs DoubleRow on a x16-scaled fp8 copy of A: contraction 256
    per pass halves the matmul count (8 instead of 16).  Phase 2 stays
    fp16: its ACT/DVE consumers (n2/bp2/W builds) run at ~550-690ns per
    [128,512] tile, which balances 4-MM groups but would starve DoubleRow
    2-MM groups.  The first-order A term in W comes from the fp16 A+I, so
    fp8 error enters only at second order (verified 1.38e-3 end to end).
  - DMA staging: one ring (sync queue) carries every input in deadline
    order -- per-engine ring FIFO serializes same-ring transfers, so the
    in-flight transfer always gets the full ~230GB/s wire:
        a8 (2x128KB), d8 (2x128KB fp8), x0, A+I (fp16), bias, x1, x2, x3
    d ships as fp8 (its A/3 term only enters W at second order via
    N2*B2); A+I is the lone fp16 matrix and rides behind x0 (needed only
    by the W build).  No link ops -- nothing blocks the DVE/ACT FIFOs.
  - PE warmup: junk matmuls on the framework const-AP region (a raw
    tensor, not a tile -> zero dependencies, first issue right at PE
    program start).  fp32 operands lower to LOW/HIGH pairs that keep the
    array busy ~213ns each, opening the HAM clock gate (~3.4us of
    sustained activity -> 2.4GHz) right around a8's landing so the expm
    runs warm.
  - expm runs i-major (outer loop over psum groups) so each group
    retires early and its consumers pipeline behind the PE; the last n2
    tile is scaled in 128-column pieces so phase-2 group j unblocks the
    moment its slice is ready.
  - Main loop: per 128-row tile, 4 accumulating fp16 matmuls of N=512
    (216ns each warm -- the PE streaming roofline) with fp32 PSUM; one
    DVE op adds bias while moving PSUM -> SBUF in fp16.  The first four
    m-tiles interleave kb-major so the kb-th matmuls land right as W
    tile kb finishes on the DVE -- the main loop starts at phase-2 end
    with no W-build stall.  Each output tile ships individually as soon
    as it is built (descriptor-gen alternates the two HWDGE queues) so
    out-traffic spreads across the main loop; the last tile ships in
    descending pieces (1/2, 1/4, 1/8, 1/8) on alternating queues so the
    teardown-gating receipt trails the last matmul by ~2us.
  - Remaining fixed overhead outside this program's control: ~6us NEFF
    prologue before the measured window and ~8us of runtime/walrus
    teardown (full 256-semaphore file reset + final barriers) inside it.
"""

import numpy as np

import concourse.bacc as bacc
import concourse.bass as bass
import concourse.mybir as mybir
import concourse.tile as tile
from concourse.bass_utils import run_bass_kernel_spmd

DIM = 512
BATCH = 16384
N_CORES = 8
XB = BATCH // N_CORES          # rows per core
P = 128                        # partitions
KT = DIM // P                  # 4 k-tiles
MT = XB // P                   # 16 m-tiles per core
XC = 4                         # m-tiles per x DMA chunk
NWARM = 23                     # PE warmup matmuls during the a8 DMA wait
SA = 16.0                      # fp8 A scale
F32 = mybir.dt.float32
F16 = mybir.dt.float16
F8 = mybir.dt.float8e4

_CACHE = {}


def build_bass():
    nc = bacc.Bacc("TRN2", target_bir_lowering=False, debug=False)

    # operands arrive pre-tiled from the host ([P, KT, ...] / chunk-major
    # for x) so every load is one contiguous-per-partition DMA
    xt_d = nc.dram_tensor("xt", [MT // XC, P, KT, P * XC], F16, kind="ExternalInput")
    a8_d = nc.dram_tensor("a8", [P, 2, 2, DIM], F8, kind="ExternalInput")
    db_d = nc.dram_tensor("db", [P, KT, DIM], F8, kind="ExternalInput")
    ai_d = nc.dram_tensor("ai", [P, KT, DIM], F16, kind="ExternalInput")
    biasr_d = nc.dram_tensor("biasr", [P, DIM], F16, kind="ExternalInput")
    out_d = nc.dram_tensor("out", [XB, DIM], F16, kind="ExternalOutput")

    AOP = mybir.AluOpType

    with tile.TileContext(nc) as tc:
        with (
            tc.tile_pool(name="const", bufs=1) as cpool,
            tc.tile_pool(name="xin", bufs=MT // XC) as xpool,
            tc.tile_pool(name="oout", bufs=6) as opool,
            tc.tile_pool(name="eps", bufs=4, space=bass.MemorySpace.PSUM) as eps,
            tc.tile_pool(name="ops", bufs=4, space=bass.MemorySpace.PSUM) as ops,
        ):
            ai_sb = cpool.tile([P, KT, DIM], F16)    # A + I
            a8_sb = cpool.tile([P, 2, 2, DIM], F8)   # 16*A, DoubleRow layout
            db_sb = cpool.tile([P, KT, DIM], F8)     # 2*d = I + A/3 (fp8)
            biasr_sb = cpool.tile([P, DIM], F16)
            xch = [
                xpool.tile([P, KT, P * XC], F16, tag="x", name=f"xc{c}")
                for c in range(MT // XC)
            ]

            # ---- PE warmup first: junk matmuls on the framework's const-AP
            # region (a raw tensor, not a tile -> the scheduler emits no
            # waits), N=128 via a stride-0 broadcast.  They issue at PE
            # program start and bridge launch-to-a8-landing so the HAM
            # clock gate opens early.  fp32 operands stream 4 cycles/col,
            # so each matmul keeps the array busy ~427ns (cold).
            c0 = nc.const_aps.aps[(F32, 0.0)]
            warm_ps = eps.tile([P, DIM], F32, tag="eps", name="warm")
            for _ in range(NWARM):
                nc.tensor.matmul(
                    warm_ps[0:1, :P],
                    c0,
                    c0.broadcast_to([P, P]),
                    start=True,
                    stop=True,
                )

            # ---- DMA kickoff ----
            # One ring (sync) carries every input in deadline order — the
            # per-engine ring FIFO serializes same-ring transfers, so the
            # in-flight transfer always gets the full wire and each lands
            # before its consumer needs it:
            #   a8(t01), a8(t23), d(t0..t3), x0, bias, x1, x2, x3
            nc.sync.dma_start(a8_sb[:, 0, :, :], a8_d[:, 0, :, :])
            nc.sync.dma_start(a8_sb[:, 1, :, :], a8_d[:, 1, :, :])
            for t in range(0, KT, 2):
                nc.sync.dma_start(db_sb[:, t : t + 2, :], db_d[:, t : t + 2, :])
            nc.sync.dma_start(xch[0][:, :, :], xt_d[0, :, :, :])
            nc.sync.dma_start(ai_sb[:, :, :], ai_d[:, :, :])
            nc.sync.dma_start(biasr_sb[:, :], biasr_d[:, :])
            for c in range(1, MT // XC):
                nc.sync.dma_start(xch[c][:, :, :], xt_d[c, :, :, :])

            # ---- expm chain (replicated) ----
            # i-major: each psum group retires early so its consumers (bp2
            # on DVE, n2 on ACT) pipeline behind the PE.
            n2_sb = cpool.tile([P, KT, DIM], F16)    # -A^2/2
            bp2_sb = cpool.tile([P, KT, DIM], F16)   # 2*B2
            m_sb = cpool.tile([P, KT, DIM], F16)     # W

            DR = mybir.MatmulPerfMode.DoubleRow
            SN = 0.5 / (SA * SA)          # psum -> -A^2/2
            SB2 = 2.0 / (24.0 * SA * SA)  # psum -> -2*A^2/24
            pss = []
            for i in range(KT):
                ps = eps.tile([P, DIM], F32, tag="eps")
                pss.append(ps)
            for i in range(KT):
                for h in range(2):
                    nc.tensor.matmul(
                        pss[i][:, :],
                        a8_sb[:, h, :, P * i : P * (i + 1)],
                        a8_sb[:, h, :, :],
                        start=(h == 0),
                        stop=(h == 1),
                        perf_mode=DR,
                    )
                # bp2 = 2*B2 in fp16; db holds 2*d so only one stt per tile
                nc.vector.scalar_tensor_tensor(
                    bp2_sb[:, i, :], pss[i][:, :], -SB2,
                    db_sb[:, i, :], AOP.mult, AOP.add,
                )
                if i == KT - 1:
                    # last tile in 128-col pieces: phase-2 group j's t=3
                    # matmul only needs columns 128j:128(j+1)
                    for j in range(KT):
                        nc.scalar.mul(
                            n2_sb[:, i, P * j : P * (j + 1)],
                            pss[i][:, P * j : P * (j + 1)], SN,
                        )
                elif i == KT - 2:
                    nc.scalar.mul(n2_sb[:, i, :P], pss[i][:, :P], SN)
                    nc.scalar.mul(n2_sb[:, i, P:], pss[i][:, P:], SN)
                else:
                    nc.scalar.mul(n2_sb[:, i, :], pss[i][:, :], SN)

            # P2 = (-A^2) @ B2; W tile i emerges right after psum group i
            # stops:  W = (A+I) - P2
            pss2 = []
            for i in range(KT):
                ps = eps.tile([P, DIM], F32, tag="eps")
                pss2.append(ps)
            for i in range(KT):
                for t in range(KT):
                    nc.tensor.matmul(
                        pss2[i][:, :],
                        n2_sb[:, t, P * i : P * (i + 1)],
                        bp2_sb[:, t, :],
                        start=(t == 0),
                        stop=(t == KT - 1),
                    )
                # psum = P2 exactly (operand scales 1/2 and 2)
                nc.vector.scalar_tensor_tensor(
                    m_sb[:, i, :], pss2[i][:, :], -1.0, ai_sb[:, i, :],
                    AOP.mult, AOP.add,
                )

            # ---- main loop: out = x @ W + bias ----
            # first four m-tiles' accumulations interleaved kb-major so the
            # kb-th matmuls land right as W tile kb finishes on the DVE —
            # the main loop starts at phase-2 end with no W-build stall
            ps01 = [
                ops.tile([P, DIM], F32, tag="out", name=f"ps0{j}")
                for j in range(XC)
            ]
            for kb in range(KT):
                for j in range(XC):
                    nc.tensor.matmul(
                        ps01[j][:, :],
                        xch[0][:, kb, P * j : P * (j + 1)],
                        m_sb[:, kb, :],
                        start=(kb == 0),
                        stop=(kb == KT - 1),
                    )
            for mi in range(MT):
                xc = xch[mi // XC]
                mo = P * (mi % XC)
                if mi < XC:
                    ps = ps01[mi]
                else:
                    ps = ops.tile([P, DIM], F32, tag="out")
                    for kb in range(KT):
                        nc.tensor.matmul(
                            ps[:, :],
                            xc[:, kb, mo : mo + P],
                            m_sb[:, kb, :],
                            start=(kb == 0),
                            stop=(kb == KT - 1),
                        )
                eng = nc.sync if (mi % 2 == 0) else nc.scalar
                if mi < MT - 1:
                    ot = opool.tile([P, DIM], F16, tag="o")
                    nc.vector.tensor_add(ot[:, :], ps[:, :], biasr_sb[:, :])
                    eng.dma_start(out_d[P * mi : P * (mi + 1), :], ot[:, :])
                else:
                    # last tile split 3/4 + 1/4: the final transfer (whose
                    # completion receipt gates teardown) is small and its
                    # DVE add is short
                    # last tile in descending pieces (1/2, 1/4, 1/8, 1/8)
                    # on alternating queues: every piece's descriptor-gen,
                    # wire time and completion receipt overlap the next
                    # piece's DVE add, so the teardown-gating receipt is
                    # only ~1us after the last add
                    ot = opool.tile([P, DIM], F16, tag="o")
                    cuts = [0, DIM // 2, 3 * DIM // 4, 7 * DIM // 8, DIM]
                    for ci in range(4):
                        lo, hi = cuts[ci], cuts[ci + 1]
                        nc.vector.tensor_add(
                            ot[:, lo:hi], ps[:, lo:hi], biasr_sb[:, lo:hi]
                        )
                        qeng = nc.sync if (ci % 2 == 0) else nc.scalar
                        qeng.dma_start(
                            out_d[P * mi : P * (mi + 1), lo:hi], ot[:, lo:hi]
                        )

    nc.compile()
    return nc


def _get_nc():
    if "nc" not in _CACHE:
        _CACHE["nc"] = build_bass()
    return _CACHE["nc"]


def _host_inputs(angles, bias):
    import ml_dtypes

    angles = np.asarray(angles, dtype=np.float32)
    bias = np.asarray(bias, dtype=np.float32)
    iu, ju = np.triu_indices(DIM, k=1)
    A = np.zeros((DIM, DIM), dtype=np.float32)
    A[iu, ju] = angles
    A[ju, iu] = -angles

    def tiled(mat):
        # [DIM, DIM] -> [P, KT, DIM] with tiled[p, t, :] = mat[128t + p, :]
        return np.ascontiguousarray(
            mat.reshape(KT, P, DIM).transpose(1, 0, 2)
        )

    def dr(mat):
        # DoubleRow interleave: [P, 2, 2, DIM] with
        # dr[kk, h, j, :] = mat[256h + 128j + kk, :]
        return np.ascontiguousarray(
            mat.reshape(2, 2, P, DIM).transpose(2, 0, 1, 3)
        )

    d2 = np.eye(DIM, dtype=np.float32) + A / np.float32(3.0)   # 2*d
    return {
        "a8": dr((A * np.float32(SA)).astype(ml_dtypes.float8_e4m3)),
        "db": tiled(d2.astype(ml_dtypes.float8_e4m3)),
        "ai": tiled((np.eye(DIM, dtype=np.float32) + A).astype(np.float16)),
        "biasr": np.ascontiguousarray(
            np.broadcast_to(bias.reshape(1, DIM), (P, DIM))
        ).astype(np.float16),
    }


def kernel(x, angles, bias, _profile=False):
    x = np.asarray(x, dtype=np.float32)
    # per-core x shards, pre-transposed and pre-tiled to chunk-major
    # [chunk, p, t, m] with element = x[512*chunk + m, 128*t + p], fp16
    xts = np.ascontiguousarray(
        x.reshape(N_CORES, MT // XC, P * XC, KT, P).transpose(0, 1, 4, 3, 2)
    ).astype(np.float16)
    shared = _host_inputs(angles, bias)
    nc = _get_nc()
    in_maps = [{"xt": xts[c], **shared} for c in range(N_CORES)]
    res = run_bass_kernel_spmd(
        nc, in_maps, list(range(N_CORES)), trace=bool(_profile)
    )
    _CACHE["last_result"] = res
    out = np.concatenate(
        [np.asarray(res.results[c]["out"]) for c in range(N_CORES)], axis=0
    )
    return out.astype(np.float32)


# revision 39
# speedup vs baseline: 1.1416x; 1.0011x over previous
"""Trainium2 Bass kernel for out = x @ expm(skew(angles)) + bias.

Measured ~36.5-37.3us median HW exec on 8 NeuronCores (prior baseline
38.2us), relative error ~1.4e-3 (gate 2e-2).

Strategy:
  - Data-parallel over the batch: x [16384, 512] split into 8 shards of
    [2048, 512]; angles/bias and the tiny expm replicated on every core.
    Host only does layout/precision marshaling (skew build, dtype casts,
    pre-tiled transposes); all linear algebra runs on-device.
  - Rotation via degree-4 Taylor with 2 matmuls of 512^3:
        psum1 = (16A)^T (16A)            (fp8e4 DoubleRow, 2 MMs/group)
        n2  = psum1/512     = -A^2/2     (fp16, ACT)
        bp2 = 2d - psum1/3072 = 2 B2     (fp16, DVE; d = I/2 + A/6)
        psum2 = n2 @ bp2    = P2         (fp16 operands, 4 MMs/group)
        W   = (A + I) - P2               (DVE; A+I shipped in fp16)
    Phase 1 runs DoubleRow on a x16-scaled fp8 copy of A: contraction 256
    per pass halves the matmul count (8 instead of 16).  Phase 2 stays
    fp16: its ACT/DVE consumers (n2/bp2/W builds) run at ~550-690ns per
    [128,512] tile, which balances 4-MM groups but would starve DoubleRow
    2-MM groups.  The first-order A term in W comes from the fp16 A+I, so
    fp8 error enters only at second order (verified 1.38e-3 end to end).
  - DMA staging: one ring (sync queue) carries every input in deadline
    order -- per-engine ring FIFO serializes same-ring transfers, so the
    in-flight transfer always gets the full ~230GB/s wire:
        a8 (2x128KB), d8 (2x128KB fp8), x0, A+I (fp16), bias, x1, x2, x3
    d ships as fp8 (its A/3 term only enters W at second order via
    N2*B2); A+I is the lone fp16 matrix and rides behind x0 (needed only
    by the W build).  No link ops -- nothing blocks the DVE/ACT FIFOs.
  - PE warmup: junk matmuls on the framework const-AP region (a raw
    tensor, not a tile -> zero dependencies, first issue right at PE
    program start).  fp32 operands lower to LOW/HIGH pairs that keep the
    array busy ~213ns each, opening the HAM clock gate (~3.4us of
    sustained activity -> 2.4GHz) right around a8's landing so the expm
    runs warm.
  - expm runs i-major (outer loop over psum groups) so each group
    retires early and its consumers pipeline behind the PE; the last n2
    tile is scaled in 128-column pieces so phase-2 group j unblocks the
    moment its slice is ready.
  - Main loop: per 128-row tile, 4 accumulating fp16 matmuls of N=512
    (216ns each warm -- the PE streaming roofline) with fp32 PSUM; one
    DVE op adds bias while moving PSUM -> SBUF in fp16.  The first four
    m-tiles interleave kb-major so the kb-th matmuls land right as W
    tile kb finishes on the DVE -- the main loop starts at phase-2 end
    with no W-build stall.  Each output tile ships individually as soon
    as it is built (descriptor-gen alternates the two HWDGE queues) so
    out-traffic spreads across the main loop; the last tile ships in
    descending pieces (1/2, 1/4, 1/8, 1/8) on alternating queues so the
    teardown-gating receipt trails the last matmul by ~2us.
  - Remaining fixed overhead outside this program's control: ~6us NEFF
    prologue before the measured window and ~8us of runtime/walrus
    teardown (full 256-semaphore file reset + final barriers) inside it.
"""

import numpy as np

import concourse.bacc as bacc
import concourse.bass as bass
import concourse.mybir as mybir
import concourse.tile as tile
from concourse.bass_utils import run_bass_kernel_spmd

DIM = 512
BATCH = 16384
N_CORES = 8
XB = BATCH // N_CORES          # rows per core
P = 128                        # partitions
KT = DIM // P                  # 4 k-tiles
MT = XB // P                   # 16 m-tiles per core
XC = 4                         # m-tiles per x DMA chunk
NWARM = 23                     # PE warmup matmuls during the a8 DMA wait
SA = 16.0                      # fp8 A scale
F32 = mybir.dt.float32
F16 = mybir.dt.float16
F8 = mybir.dt.float8e4

_CACHE = {}


def build_bass():
    nc = bacc.Bacc("TRN2", target_bir_lowering=False, debug=False)

    # operands arrive pre-tiled from the host ([P, KT, ...] / chunk-major
    # for x) so every load is one contiguous-per-partition DMA
    xt_d = nc.dram_tensor("xt", [MT // XC, P, KT, P * XC], F16, kind="ExternalInput")
    a8_d = nc.dram_tensor("a8", [P, 2, 2, DIM], F8, kind="ExternalInput")
    db_d = nc.dram_tensor("db", [P, KT, DIM], F8, kind="ExternalInput")
    ai_d = nc.dram_tensor("ai", [P, KT, DIM], F16, kind="ExternalInput")
    biasr_d = nc.dram_tensor("biasr", [P, DIM], F16, kind="ExternalInput")
    out_d = nc.dram_tensor("out", [XB, DIM], F16, kind="ExternalOutput")

    AOP = mybir.AluOpType

    with tile.TileContext(nc) as tc:
        with (
            tc.tile_pool(name="const", bufs=1) as cpool,
            tc.tile_pool(name="xin", bufs=MT // XC) as xpool,
            tc.tile_pool(name="oout", bufs=6) as opool,
            tc.tile_pool(name="eps", bufs=4, space=bass.MemorySpace.PSUM) as eps,
            tc.tile_pool(name="ops", bufs=4, space=bass.MemorySpace.PSUM) as ops,
        ):
            ai_sb = cpool.tile([P, KT, DIM], F16)    # A + I
            a8_sb = cpool.tile([P, 2, 2, DIM], F8)   # 16*A, DoubleRow layout
            db_sb = cpool.tile([P, KT, DIM], F8)     # 2*d = I + A/3 (fp8)
            biasr_sb = cpool.tile([P, DIM], F16)
            xch = [
                xpool.tile([P, KT, P * XC], F16, tag="x", name=f"xc{c}")
                for c in range(MT // XC)
            ]

            # ---- PE warmup first: junk matmuls on the framework's const-AP
            # region (a raw tensor, not a tile -> the scheduler emits no
            # waits), N=128 via a stride-0 broadcast.  They issue at PE
            # program start and bridge launch-to-a8-landing so the HAM
            # clock gate opens early.  fp32 operands stream 4 cycles/col,
            # so each matmul keeps the array busy ~427ns (cold).
            c0 = nc.const_aps.aps[(F32, 0.0)]
            # 1-element zero-dependency DVE op: dispatches at DVE program
            # start so the first real stt (bp2[0], which gates phase 2)
            # issues from a warm queue instead of paying cold-dispatch lag
            nc.vector.tensor_scalar_mul(biasr_sb[0:1, 0:1], c0[0:1, 0:1], 0.0)
            warm_ps = eps.tile([P, DIM], F32, tag="eps", name="warm")
            for _ in range(NWARM):
                nc.tensor.matmul(
                    warm_ps[0:1, :P],
                    c0,
                    c0.broadcast_to([P, P]),
                    start=True,
                    stop=True,
                )

            # ---- DMA kickoff ----
            # One ring (sync) carries every input in deadline order — the
            # per-engine ring FIFO serializes same-ring transfers, so the
            # in-flight transfer always gets the full wire and each lands
            # before its consumer needs it:
            #   a8(t01), a8(t23), d(t0..t3), x0, bias, x1, x2, x3
            nc.sync.dma_start(a8_sb[:, 0, :, :], a8_d[:, 0, :, :])
            nc.sync.dma_start(a8_sb[:, 1, :, :], a8_d[:, 1, :, :])
            for t in range(0, KT, 2):
                nc.sync.dma_start(db_sb[:, t : t + 2, :], db_d[:, t : t + 2, :])
            nc.sync.dma_start(xch[0][:, :, :], xt_d[0, :, :, :])
            nc.sync.dma_start(ai_sb[:, :, :], ai_d[:, :, :])
            nc.sync.dma_start(biasr_sb[:, :], biasr_d[:, :])
            for c in range(1, MT // XC):
                nc.sync.dma_start(xch[c][:, :, :], xt_d[c, :, :, :])

            # ---- expm chain (replicated) ----
            # i-major: each psum group retires early so its consumers (bp2
            # on DVE, n2 on ACT) pipeline behind the PE.
            n2_sb = cpool.tile([P, KT, DIM], F16)    # -A^2/2
            bp2_sb = cpool.tile([P, KT, DIM], F16)   # 2*B2
            m_sb = cpool.tile([P, KT, DIM], F16)     # W

            DR = mybir.MatmulPerfMode.DoubleRow
            SN = 0.5 / (SA * SA)          # psum -> -A^2/2
            SB2 = 2.0 / (24.0 * SA * SA)  # psum -> -2*A^2/24
            pss = []
            for i in range(KT):
                ps = eps.tile([P, DIM], F32, tag="eps")
                pss.append(ps)
            for i in range(KT):
                for h in range(2):
                    nc.tensor.matmul(
                        pss[i][:, :],
                        a8_sb[:, h, :, P * i : P * (i + 1)],
                        a8_sb[:, h, :, :],
                        start=(h == 0),
                        stop=(h == 1),
                        perf_mode=DR,
                    )
                # bp2 = 2*B2 in fp16; db holds 2*d so only one stt per tile
                nc.vector.scalar_tensor_tensor(
                    bp2_sb[:, i, :], pss[i][:, :], -SB2,
                    db_sb[:, i, :], AOP.mult, AOP.add,
                )
                if i == KT - 1:
                    # last tile in 128-col pieces: phase-2 group j's t=3
                    # matmul only needs columns 128j:128(j+1)
                    for j in range(KT):
                        nc.scalar.mul(
                            n2_sb[:, i, P * j : P * (j + 1)],
                            pss[i][:, P * j : P * (j + 1)], SN,
                        )
                elif i == KT - 2:
                    nc.scalar.mul(n2_sb[:, i, :P], pss[i][:, :P], SN)
                    nc.scalar.mul(n2_sb[:, i, P:], pss[i][:, P:], SN)
                else:
                    nc.scalar.mul(n2_sb[:, i, :], pss[i][:, :], SN)

            # P2 = (-A^2) @ B2; W tile i emerges right after psum group i
            # stops:  W = (A+I) - P2
            pss2 = []
            for i in range(KT):
                ps = eps.tile([P, DIM], F32, tag="eps")
                pss2.append(ps)
            for i in range(KT):
                for t in range(KT):
                    nc.tensor.matmul(
                        pss2[i][:, :],
                        n2_sb[:, t, P * i : P * (i + 1)],
                        bp2_sb[:, t, :],
                        start=(t == 0),
                        stop=(t == KT - 1),
                    )
                # psum = P2 exactly (operand scales 1/2 and 2)
                nc.vector.scalar_tensor_tensor(
                    m_sb[:, i, :], pss2[i][:, :], -1.0, ai_sb[:, i, :],
                    AOP.mult, AOP.add,
                )

            # ---- main loop: out = x @ W + bias ----
            # first four m-tiles' accumulations interleaved kb-major so the
            # kb-th matmuls land right as W tile kb finishes on the DVE —
            # the main loop starts at phase-2 end with no W-build stall
            ps01 = [
                ops.tile([P, DIM], F32, tag="out", name=f"ps0{j}")
                for j in range(XC)
            ]
            for kb in range(KT):
                for j in range(XC):
                    nc.tensor.matmul(
                        ps01[j][:, :],
                        xch[0][:, kb, P * j : P * (j + 1)],
                        m_sb[:, kb, :],
                        start=(kb == 0),
                        stop=(kb == KT - 1),
                    )
            for mi in range(MT):
                xc = xch[mi // XC]
                mo = P * (mi % XC)
                if mi < XC:
                    ps = ps01[mi]
                else:
                    ps = ops.tile([P, DIM], F32, tag="out")
                    for kb in range(KT):
                        nc.tensor.matmul(
                            ps[:, :],
                            xc[:, kb, mo : mo + P],
                            m_sb[:, kb, :],
                            start=(kb == 0),
                            stop=(kb == KT - 1),
                        )
                eng = nc.sync if (mi % 2 == 0) else nc.scalar
                if mi < MT - 1:
                    ot = opool.tile([P, DIM], F16, tag="o")
                    nc.vector.tensor_add(ot[:, :], ps[:, :], biasr_sb[:, :])
                    eng.dma_start(out_d[P * mi : P * (mi + 1), :], ot[:, :])
                else:
                    # last tile split 3/4 + 1/4: the final transfer (whose
                    # completion receipt gates teardown) is small and its
                    # DVE add is short
                    # last tile in descending pieces (1/2, 1/4, 1/8, 1/8)
                    # on alternating queues: every piece's descriptor-gen,
                    # wire time and completion receipt overlap the next
                    # piece's DVE add, so the teardown-gating receipt is
                    # only ~1us after the last add
                    ot = opool.tile([P, DIM], F16, tag="o")
                    cuts = [0, DIM // 2, 3 * DIM // 4, 7 * DIM // 8, DIM]
                    for ci in range(4):
                        lo, hi = cuts[ci], cuts[ci + 1]
                        nc.vector.tensor_add(
                            ot[:, lo:hi], ps[:, lo:hi], biasr_sb[:, lo:hi]
                        )
                        qeng = nc.sync if (ci % 2 == 0) else nc.scalar
                        qeng.dma_start(
                            out_d[P * mi : P * (mi + 1), lo:hi], ot[:, lo:hi]
                        )

    nc.compile()
    return nc


def _get_nc():
    if "nc" not in _CACHE:
        _CACHE["nc"] = build_bass()
    return _CACHE["nc"]


def _host_inputs(angles, bias):
    import ml_dtypes

    angles = np.asarray(angles, dtype=np.float32)
    bias = np.asarray(bias, dtype=np.float32)
    iu, ju = np.triu_indices(DIM, k=1)
    A = np.zeros((DIM, DIM), dtype=np.float32)
    A[iu, ju] = angles
    A[ju, iu] = -angles

    def tiled(mat):
        # [DIM, DIM] -> [P, KT, DIM] with tiled[p, t, :] = mat[128t + p, :]
        return np.ascontiguousarray(
            mat.reshape(KT, P, DIM).transpose(1, 0, 2)
        )

    def dr(mat):
        # DoubleRow interleave: [P, 2, 2, DIM] with
        # dr[kk, h, j, :] = mat[256h + 128j + kk, :]
        return np.ascontiguousarray(
            mat.reshape(2, 2, P, DIM).transpose(2, 0, 1, 3)
        )

    d2 = np.eye(DIM, dtype=np.float32) + A / np.float32(3.0)   # 2*d
    return {
        "a8": dr((A * np.float32(SA)).astype(ml_dtypes.float8_e4m3)),
        "db": tiled(d2.astype(ml_dtypes.float8_e4m3)),
        "ai": tiled((np.eye(DIM, dtype=np.float32) + A).astype(np.float16)),
        "biasr": np.ascontiguousarray(
            np.broadcast_to(bias.reshape(1, DIM), (P, DIM))
        ).astype(np.float16),
    }


def kernel(x, angles, bias, _profile=False):
    x = np.asarray(x, dtype=np.float32)
    # per-core x shards, pre-transposed and pre-tiled to chunk-major
    # [chunk, p, t, m] with element = x[512*chunk + m, 128*t + p], fp16
    xts = np.ascontiguousarray(
        x.reshape(N_CORES, MT // XC, P * XC, KT, P).transpose(0, 1, 4, 3, 2)
    ).astype(np.float16)
    shared = _host_inputs(angles, bias)
    nc = _get_nc()
    in_maps = [{"xt": xts[c], **shared} for c in range(N_CORES)]
    res = run_bass_kernel_spmd(
        nc, in_maps, list(range(N_CORES)), trace=bool(_profile)
    )
    _CACHE["last_result"] = res
    out = np.concatenate(
        [np.asarray(res.results[c]["out"]) for c in range(N_CORES)], axis=0
    )
    return out.astype(np.float32)
